# revision 61
# baseline (speedup 1.0000x reference)
"""MedianPool2d (3x3, stride 1, zero-pad 1) Trainium2 Bass kernel.

Full input x: (8, 64, 256, 256) fp32.  Sharding: pure data parallel over
batch -> core i processes x[i] (64, 256, 256).

Per-core layout: 128 SBUF partitions = (h, c) with h in {0,1} picking the
top/bottom 128-row half of the image and c the channel.  Rows are stored
in fp16, column-DEINTERLEAVED: row = [E | O] where E[t] = padded col 2t
(129 wide, E[0] is the left zero-pad col) and O[t] = padded col 2t+1
(129 wide, O[128] is the right zero-pad col).

Median of 9 = med3(max3(column lo), med3(column me), min3(column hi)).
The deinterleaved layout makes every horizontal even/odd pair op a
contiguous-block op, so ALL DVE tensor_tensor instructions are fp16 with
unit-stride last dims -> they hit the DVE 2x_1p performance mode
(2 elem/cycle/lane).  Elems/px on DVE: vertical 5.04 (shared row pairs)
+ max3 1.50 + min3 1.50 + med3 3.01 + final med3 4 = 15.06 -> 7.53
cycles/px, ~2x faster than the fp32 15 op/px variant.

The Activation engine does fp32->fp16 cast + deinterleave on load,
fp16->fp32 cast + re-interleave on store, and the zero-pad writes (it
cannot do two-tensor min/max, but casts/copies run there for free
alongside the DVE).  The neuron backend rejects TensorTensor /
TensorScalarPtr on GPSIMD, so the DVE does all min/max.

Instruction fusion with raw access patterns cuts the DVE to 16
instructions per chunk: even/odd completions fuse on a parity dim
(broadcast pair operands + min/max commutativity), and co-allocating
[Lo|Me|Hi|tEO] / [PA|u|PC|v] as single tiles lets adjacent same-op
pairs fuse on a selector dim ([PA;v] = max over {Lo,Me} halves,
[u;PC] = min over {Me,Hi}).  HW limits: TensorTensor APs allow at
most 3 FREE dims (4D+ fusions are rejected by codegen -- and small
probes can pass deceptively when dims collapse), and negative-stride
selector dims pass codegen but fault the device at runtime, so only
positive-stride 3-free-dim fusions ship.  Final-stage temporaries
alias only slots whose next-chunk writers come late (mT dedicated,
mU->P4, mV->A) so the next chunk's pair ops interleave into the
final med3 dependency chain and hide semaphore latency.

fp16 keeps the min/max selection network exact up to input rounding:
max rel err ~1.7e-2 (at small-magnitude medians), l2 rel err ~2e-4
(gate is 2e-2).

Chunks of R=16 rows pipeline DMA -> ACT -> DVE -> ACT -> DMA on split
DGE queues; first/last chunks are tapered to shrink pipeline
fill/drain; the last chunk's final op writes fp32 interleaved
directly from the DVE (skipping the ACT hop) and its two half-stores
are fused into one DMA via a 4D DRAM-side pattern
(p = h*64+c -> out[c, h*HH+r, w]), shortening the drain path.

PE/ACT offload (the "f16o" variant): three plane-ops per chunk move
off the DVE to the otherwise idle PE + spare ACT capacity via the
lattice identity  min,max = (a+b)/2 -+ |a-b|/2:
identity-scaled matmuls (weights 0.5I/I/-I/2I staged in SBUF)
accumulate s = (a+b)/2 and d = a-b into bank-aligned PSUM tiles, ACT
computes h = |d/2| (Abs is in every ACT table set -> no table-switch
cost) and evacuates s-h / s+2h as fp16 min/max after -h / +2h matmul
accumulations.  Offloaded this way:
 1. the vertical pair stage Pm/PM (chain issued one chunk ahead --
    x16 is prefetched -- so the DVE never waits on it), and
 2. the final-stage CE mT/mU = min/max(A,Bt) ("chain2"), with the
    remaining mV/ot DVE ops DEFERRED one chunk (software pipelining:
    stage2(k) runs after stage1(k+1), giving the chain a full chunk
    of lead).  This required splitting CT12 into t12 + double-
    buffered Ct, un-aliasing mU, and emitting stage2 BEFORE the next
    chunk's mT-tag writers (else a WAR/queue-order deadlock).
The input cast/deinterleave runs on GPSIMD tensor_copy (HW-verified
exact incl. the negative-stride parity AP), freeing ACT for the chain
evacuations.  Chains are restricted to the middle chunks (taper
2,6,8,12x8,8,6,2; PmM chains k>=3, chain2 k=3..9): the tiny edge
chunks cannot hide the chain cold-start (PE p-state ramp + serial
PE->ACT tail).  Tuned windows: PmM chains k>=3, chain2 k=3..10.
Tried and WORSE (all reverted): extending chains to mV/ot (341us:
serialized chain exceeds the one-chunk lead, ACT per-instr PSUM
overheads explode); double-buffering mT2/mU2 +/- reordering stage2
(the single-buffer WAR constraints steer Tile's list scheduler into
a better order); dei on ACT for early chunks; wider off2 windows;
bigger head tapers.  Also ruled out: TensorTensor on GPSIMD (neuron
backend rejects it), custom DVE uops (2 streams / 1x rate cap), and
further PE offload (tEO/Hi would need 258-el per-row matmuls that
push PE past its per-chunk window; identity-matmul costs ~2.3x what
the DVE saves per element).

TimelineSim: 245371 ns/core (DVE busy ~230us, was ~267us) vs 279064
for the pure-DVE f16 variant and 546694 for the fp32 baseline.
Chunk 0's dei runs on the then-idle DVE itself (halves the fill gap;
doing the same for the last chunk lengthens the drain instead);
o32 is triple-buffered (breaks a store WAR on the drain path; triple-
buffering any other pool makes the schedule worse).  Taper (2,6,8,12x9,2,2) with chain2 on
all nine 12-row middles and a (12,2,2) tail: the 12-row chunk 11
joins both chains and the tiny (2,2) tail drains fast.  ACT runs ~97% busy through the middle
phase (PmM + chain2 evacuations + interleave): it is the co-binding
engine, which is why PmM evacs land ~0.5us late each chunk and why
every attempt to move the interleave or dei halves onto/off ACT
perturbs the cadence and loses more in gaps than it saves (all such
variants measured worse and reverted; evacuations cannot leave ACT --
GPSIMD has no PSUM port).  Remaining ~15us of DVE idle: ~6us
DMA-bound fill (DMA issue latency + serial GPSIMD dei of chunk 0),
~3.4us drain, ~5us chain-latency gaps at the off2 window edges.
HW-validated end to end: l2 rel err 3.461e-04 (gate 2e-2); each
kernel() call is spot-checked against the exact median on 400k random
pixels with retry + pure-f16 + fp32-exact fallbacks (a rare cold-start
HW corruption was observed once in ~10 runs before the guard).
"""

import numpy as np

B, C, H, W = 8, 64, 256, 256
NCORES = 8
HH = H // 2           # rows per half-strip
WP = W + 2            # 258 padded width
NE = WP // 2          # 129 = evens block width (incl. left zero col)

_CACHE = {}


def _build_f16(R=16, taper=(2, 6, 16, 16, 16, 16, 16, 16, 16, 6, 2),
               fuse=True, offload=False, off0=3, minoff=16, keep_tail=0,
               off2_win=(3, 2)):
    if offload and taper == (2, 6, 16, 16, 16, 16, 16, 16, 16, 6, 2):
        # smaller DVE-computed edge chunks so the DVE PmM tile for the
        # non-offloaded chunks stays small (SBUF) while chains cover the
        # six full 16-row chunks
        taper = (2, 6, 8, 16, 16, 16, 16, 16, 16, 8, 6, 2)
    import concourse.bacc as bacc
    import concourse.mybir as mybir
    from concourse.tile import TileContext
    from concourse.ap import AP

    MIN = mybir.AluOpType.min
    MAX = mybir.AluOpType.max
    f32 = mybir.dt.float32
    f16 = mybir.dt.float16
    ABS = mybir.ActivationFunctionType.Abs

    # chunk row counts: small first/last chunks shrink pipeline fill/drain
    if isinstance(taper, (list, tuple)):
        chunks = list(taper)
    elif taper and taper < R:
        mid = HH - 2 * taper
        assert mid % R == 0
        chunks = [taper] + [R] * (mid // R) + [taper]
    else:
        chunks = [R] * (HH // R)
    assert sum(chunks) == HH and all(c % 2 == 0 for c in chunks)
    K = len(chunks)

    nc = bacc.Bacc("TRN2", name="median_pool2d_f16")
    x = nc.dram_tensor("x", [C, H, W], f32, kind="ExternalInput")
    out = nc.dram_tensor("out", [C, H, W], f32, kind="ExternalOutput")
    xg = x.ap()
    og = out.ap()
    if offload:
        # 4 identity blocks scaled {0.5, 1, -1, 2} for PE elementwise combines
        wid = nc.dram_tensor("wid", [128, 512], f16, kind="ExternalInput")

    with TileContext(nc) as tc:
        with (
            tc.tile_pool(name="io_in", bufs=2) as in_pool,
            tc.tile_pool(name="x16p", bufs=2) as x16_pool,
            tc.tile_pool(name="work", bufs=1) as w_pool,
            tc.tile_pool(name="otp", bufs=2) as ot_pool,
            tc.tile_pool(name="io_out", bufs=3) as out_pool,
            tc.tile_pool(name="wgt", bufs=1) as wgt_pool,
            tc.psum_pool(name="pep", bufs=1) as ps_pool,
        ):
            if offload:
                wsb = wgt_pool.tile([128, 512], f16, name="wsb")
                wH = wsb[:, 0:128]     # 0.5*I
                wI = wsb[:, 128:256]   # I
                wN = wsb[:, 256:384]   # -I
                w2 = wsb[:, 384:512]   # 2*I
            x16rs = {}
            r0s = [sum(chunks[:i]) for i in range(K)]

            def zcopy(out_ap, pslice=slice(0, 128)):
                """Zero a region on ACT via const-0 broadcast copy (keeps
                all x16 writes on one engine: no cross-engine WAW stalls)."""
                z = nc.const_aps.scalar_like(0.0, out_ap.tensor.ap()[:, 0:1])
                z = z[pslice]
                while z.ndim < out_ap.ndim:
                    z = z.unsqueeze(1)
                nc.scalar.copy(out=out_ap, in_=z.broadcast_to(out_ap.shape))

            def load_dei(k):
                """DMA fp32 chunk k + ACT cast/deinterleave into x16."""
                r0, R = r0s[k], chunks[k]
                x32 = in_pool.tile([128, (R + 2) * W], f32, name="x32",
                                   tag="x32")
                x32r = x32.rearrange("p (r w) -> p r w", w=W)

                x16 = x16_pool.tile([128, (R + 2) * WP], f16, name="x16",
                                    tag="x16")
                x16r = x16.rearrange("p (r w) -> p r w", w=WP)

                # zero writes first: no DMA dependency -> ACT does them
                # while the DMA is still in flight (and chunk 0's zcopy
                # triggers the one-time ACT table load at t~0)
                zcopy(x16r[:, :, 0:WP:WP - 1])
                if k == 0:
                    zcopy(x16r[0:64, 0:1, :], slice(0, 64))
                elif k == K - 1:
                    zcopy(x16r[64:128, R + 1:R + 2, :], slice(64, 128))

                # top half on the SP DGE queue, bottom half on the ACT DGE
                # queue: the two descriptor generations run in parallel
                if k == 0:
                    nc.sync.dma_start(out=x32r[0:64, 1:R + 2, :],
                                      in_=xg[:, 0:R + 1, :])
                else:
                    nc.sync.dma_start(out=x32r[0:64, :, :],
                                      in_=xg[:, r0 - 1:r0 + R + 1, :])
                if k == K - 1:
                    nc.scalar.dma_start(out=x32r[64:128, 0:R + 1, :],
                                        in_=xg[:, HH + r0 - 1:H, :])
                else:
                    nc.scalar.dma_start(out=x32r[64:128, :, :],
                                        in_=xg[:, HH + r0 - 1:HH + r0 + R + 1, :])

                def dei(pslice, rows):
                    # E: orig odd cols -> x16[,,1:129]; O: orig even -> 129:257
                    if fuse:
                        # one copy: in reads [par: E(odd cols)/O(even)]
                        # via a (-1)-stride parity dim + stride-2 col dim.
                        # On the offload variant this runs on the otherwise
                        # idle GPSIMD (cast+strided copy HW-verified exact),
                        # freeing ACT capacity for the PE-chain evacuations.
                        b = x32r[pslice, rows, :]
                        nrows = b.shape[1]
                        in_ = AP(tensor=b.tensor, offset=b.offset + 1,
                                 ap=[list(b.ap[0]), [W, nrows], [-1, 2],
                                     [2, 128]])
                        if offload and k == 0:
                            # pipeline edges: the DVE is idle waiting on
                            # exactly this data -- doing the cast itself
                            # (2x copy) beats the GPSIMD queue hop and the
                            # serialization of the two half-copies
                            eng = nc.vector.tensor_copy
                        elif offload:
                            eng = nc.gpsimd.tensor_copy
                        else:
                            eng = lambda out, in_: nc.scalar.copy(out=out,
                                                                  in_=in_)
                        eng(out=x16r[pslice, rows, 1:WP - 1], in_=in_)
                    else:
                        nc.scalar.copy(out=x16r[pslice, rows, 1:NE],
                                       in_=x32r[pslice, rows, 1:W:2])
                        nc.scalar.copy(out=x16r[pslice, rows, NE:WP - 1],
                                       in_=x32r[pslice, rows, 0:W:2])

                if k == 0:
                    dei(slice(0, 64), slice(1, R + 2))
                    dei(slice(64, 128), slice(0, R + 2))
                elif k == K - 1:
                    dei(slice(0, 64), slice(0, R + 2))
                    dei(slice(64, 128), slice(0, R + 1))
                else:
                    dei(slice(0, 128), slice(0, R + 2))
                x16rs[k] = x16r

            pmms = {}
            MM = nc.tensor.matmul if offload else None

            def pe_pair(k):
                """PE+ACT compute PmM (vertical pair min/max) for chunk k:
                s = (b0+b1)/2 and d = b0-b1 via +-identity matmuls into PSUM,
                h = |d|/2 on ACT, then Pm = s-h / PM = s+h by accumulating
                -h / +2h and evacuating each to SBUF fp16.  Issued one chunk
                ahead (x16 is prefetched), so the DVE never waits on it."""
                Rk = chunks[k]
                Rhk = Rk // 2
                x16rk = x16rs[k]
                t = w_pool.tile([128, 2 * Rhk * WP], f16, name="PmMo",
                                tag="PmMo")
                PmM = t.rearrange("p (s r w) -> p s r w", r=Rhk, w=WP)
                for g0 in range(0, Rhk, 2):
                    gr = min(2, Rhk - g0)
                    s_ps = ps_pool.tile([128, gr * 512], f32, name="s_ps",
                                        tag="s_ps")
                    d_ps = ps_pool.tile([128, gr * 512], f32, name="d_ps",
                                        tag="d_ps")
                    s3 = s_ps.rearrange("p (r w) -> p r w", w=512)
                    d3 = d_ps.rearrange("p (r w) -> p r w", w=512)
                    h = w_pool.tile([128, gr * WP], f16, name="hh", tag="hh")
                    h3 = h.rearrange("p (r w) -> p r w", w=WP)
                    for r in range(gr):
                        b0r = x16rk[:, 1 + 2 * (g0 + r), :]
                        b1r = x16rk[:, 2 + 2 * (g0 + r), :]
                        MM(s3[:, r, 0:WP], wH, b0r, start=True, stop=False)
                        MM(s3[:, r, 0:WP], wH, b1r, start=False, stop=False,
                           skip_group_check=True)
                        MM(d3[:, r, 0:WP], wI, b0r, start=True, stop=False)
                        MM(d3[:, r, 0:WP], wN, b1r, start=False, stop=True,
                           skip_group_check=True)
                    nc.scalar.activation(out=h3[:], in_=d3[:, :, 0:WP],
                                         func=ABS, scale=0.5)
                    for r in range(gr):
                        MM(s3[:, r, 0:WP], wN, h3[:, r, :], start=False,
                           stop=False, skip_group_check=True)
                    nc.scalar.copy(out=PmM[:, 0, g0:g0 + gr, :],
                                   in_=s3[:, :, 0:WP])
                    for r in range(gr):
                        MM(s3[:, r, 0:WP], w2, h3[:, r, :], start=False,
                           stop=True, skip_group_check=True)
                    nc.scalar.copy(out=PmM[:, 1, g0:g0 + gr, :],
                                   in_=s3[:, :, 0:WP])
                pmms[k] = PmM

            OFF0 = off0

            def off_k(k):
                # offload only full-size chunks, and not before chunk OFF0:
                # the tiny taper chunks at the pipeline edges (and the cold
                # PE/ACT chain at startup) cannot hide the chain latency
                return (offload and chunks[k] >= minoff and k >= OFF0
                        and k < K - keep_tail)

            mTUs = {}
            cts = {}
            pend = []

            off2_lo, off2_hi = off2_win

            def off2(k):
                # final-stage mT/mU offload: middle chunks only (mV/ot are
                # deferred one chunk, so the edges keep the inline path)
                return (offload and off2_lo <= k < K - off2_hi
                        and chunks[k] >= 12)

            def chain2(k, A_t, Bt_t):
                '''PE+ACT compute mT = min(A,Bt), mU = max(A,Bt) via
                mT,mU = (A+Bt)/2 -+ |A-Bt|/2.  Issued after stage2(k-1):
                consumed by stage2(k) a full chunk later, so the PE->ACT
                chain latency (incl. cold p-state) hides under stage1(k+1).
                '''
                n = chunks[k] * W
                mT2 = w_pool.tile([128, n], f16, name="mT2", tag="mT")
                mU2 = w_pool.tile([128, n], f16, name="mU2", tag="mU2")
                for g0 in range(0, n, 1024):
                    gl = min(1024, n - g0)
                    s2 = ps_pool.tile([128, gl], f32, name="s2", tag="s2")
                    d2 = ps_pool.tile([128, gl], f32, name="d2", tag="d2")
                    h2 = w_pool.tile([128, gl], f16, name="h2", tag="h2")
                    for b0 in range(0, gl, 512):
                        bl = min(512, gl - b0)
                        sl = slice(g0 + b0, g0 + b0 + bl)
                        bsl = slice(b0, b0 + bl)
                        MM(s2[:, bsl], wH, A_t[:, sl], start=True, stop=False)
                        MM(s2[:, bsl], wH, Bt_t[:, sl], start=False,
                           stop=False, skip_group_check=True)
                        MM(d2[:, bsl], wI, A_t[:, sl], start=True, stop=False)
                        MM(d2[:, bsl], wN, Bt_t[:, sl], start=False,
                           stop=True, skip_group_check=True)
                    nc.scalar.activation(out=h2[:], in_=d2[:], func=ABS,
                                         scale=0.5)
                    for b0 in range(0, gl, 512):
                        bl = min(512, gl - b0)
                        MM(s2[:, b0:b0 + bl], wN, h2[:, b0:b0 + bl],
                           start=False, stop=False, skip_group_check=True)
                    nc.scalar.copy(out=mT2[:, g0:g0 + gl], in_=s2[:])
                    for b0 in range(0, gl, 512):
                        bl = min(512, gl - b0)
                        MM(s2[:, b0:b0 + bl], w2, h2[:, b0:b0 + bl],
                           start=False, stop=True, skip_group_check=True)
                    nc.scalar.copy(out=mU2[:, g0:g0 + gl], in_=s2[:])
                mTUs[k] = (mT2, mU2)

            def stage2(j):
                '''Deferred mV/ot for off2 chunk j: DVE ops issued after
                stage1(j+1), reading the PE/ACT-produced mT/mU.'''
                TT = nc.vector.tensor_tensor
                rj, Rj = r0s[j], chunks[j]
                n = Rj * W
                mT2, mU2 = mTUs.pop(j)
                Ct_j = cts.pop(j)
                mVt = w_pool.tile([128, n], f16, name="mV2", tag="t12")
                ott = ot_pool.tile([128, n], f16, name="ot", tag="ot")
                otr = ott.rearrange("p (r w) -> p r w", w=W)
                o32 = out_pool.tile([128, n], f32, name="o32", tag="o32")
                o32r = o32.rearrange("p (r w) -> p r w", w=W)
                ob = o32r[:, :, :]
                oap = AP(tensor=ob.tensor, offset=ob.offset,
                         ap=[list(ob.ap[0]), [W, Rj], [1, 2], [2, 128]])
                TT(out=mVt[:], in0=mU2[:], in1=Ct_j[:], op=MIN)
                TT(out=ott[:], in0=mT2[:], in1=mVt[:], op=MAX)
                nc.scalar.copy(out=oap, in_=otr[:])
                nc.sync.dma_start(out=og[:, rj:rj + Rj, :], in_=o32r[0:64])
                nc.scalar.dma_start(out=og[:, HH + rj:HH + rj + Rj, :],
                                    in_=o32r[64:128])

            load_dei(0)
            if off_k(0):
                pe_pair(0)
            for k in range(K):
                r0, R = r0s[k], chunks[k]
                Rh = R // 2
                # prefetch + deinterleave next chunk BEFORE this chunk's
                # out-interleave, so ACT never stalls the next DVE chunk
                if k + 1 < K:
                    load_dei(k + 1)
                    if offload and k == 0:
                        # weights for the PE identity matmuls; behind the
                        # first two chunks' input DMAs (first consumer is
                        # chunk 3's PmM chain, issued in iteration 2)
                        nc.sync.dma_start(out=wsb[:], in_=wid.ap())
                    if off_k(k + 1):
                        pe_pair(k + 1)
                x16r = x16rs.pop(k)

                # ---------- vertical sort3 (shared row pairs) ----------
                def wt(name, rows, width, tag=None):
                    t = w_pool.tile([128, rows * width], f16, name=name,
                                    tag=tag or name)
                    return t.rearrange("p (r w) -> p r w", w=width)

                def wt4(name, sel, rows, width, tag=None):
                    t = w_pool.tile([128, sel * rows * width], f16, name=name,
                                    tag=tag or name)
                    return t.rearrange("p (s r w) -> p s r w", r=rows, w=width)

                # PmM: [Pm | PM] in one tile; LMHT: [Lo | Me | Hi | tEO]
                LMHT = wt4("LMHT", 4, R, WP)
                TT = nc.vector.tensor_tensor
                if off_k(k):
                    PmM = pmms.pop(k)
                else:
                    PmM = wt4("PmM", 2, Rh, WP)
                    b0 = x16r[:, 1:R + 1:2, :]
                    b1 = x16r[:, 2:R + 2:2, :]
                    TT(out=PmM[:, 0], in0=b0, in1=b1, op=MIN)
                    TT(out=PmM[:, 1], in0=b0, in1=b1, op=MAX)

                # a2: third element rows 2i+3*par (parity-fused); sel dim
                # broadcasts it over the {Lo, tEO} pair
                xb = x16r[:, 0:R + 2, :]
                pdim = list(xb.ap[0])
                a2 = AP(tensor=xb.tensor, offset=xb.offset,
                        ap=[pdim, [2 * WP, Rh], [3 * WP, 2], [1, WP]])

                def bc(sel):
                    # PmM[sel] with parity broadcast: operand for fused rows
                    tb = PmM[:, sel]
                    return AP(tensor=tb.tensor, offset=tb.offset,
                              ap=[list(tb.ap[0]), [WP, Rh], [0, 2], [1, WP]])

                pm_pd = list(PmM[:, 0].ap[0])
                lm_pd = list(LMHT[:, 0].ap[0])
                # hardware TensorTensor APs allow at most 3 free dims, so
                # Lo/tEO stay separate (parity-fused) instructions
                TT(out=LMHT[:, 0], in0=a2, in1=bc(0), op=MIN)
                TT(out=LMHT[:, 3], in0=a2, in1=bc(1), op=MIN)
                # Hi = max(a2, PM); Me = max(Pm, tEO)
                TT(out=LMHT[:, 2], in0=a2, in1=bc(1), op=MAX)
                TT(out=LMHT[:, 1], in0=bc(0), in1=LMHT[:, 3], op=MAX)

                # ---------- horizontal merge, deinterleaved blocks --------
                # P4: [PA | PC | u | v]
                P4 = wt4("P4", 4, R, NE)
                Ct_t = w_pool.tile([128, R * W], f16, name="Ct",
                                   tag=("Ct%d" % (k % 2)) if offload
                                   else "Ct")
                Ct = Ct_t.rearrange("p (r w) -> p r w", w=W)
                t12 = wt("t12", R, W)
                A_t = w_pool.tile([128, R * W], f16, name="A", tag="A")
                A = A_t.rearrange("p (r w) -> p r w", w=W)
                Bt_t = w_pool.tile([128, R * W], f16, name="Bt", tag="Bt")
                Bt = Bt_t.rearrange("p (r w) -> p r w", w=W)

                def lmh_pair(s0, step, col0):
                    # LMHT sels {s0, s0+step}, cols col0:col0+NE
                    return AP(tensor=LMHT.tensor,
                              offset=LMHT[:, s0].offset + col0,
                              ap=[lm_pd, [step * R * WP, 2], [WP, R],
                                  [1, NE]])

                p4_pd = list(P4[:, 0].ap[0])

                def p4_out(s0, step):
                    return AP(tensor=P4.tensor, offset=P4[:, s0].offset,
                              ap=[p4_pd, [step * R * NE, 2], [NE, R],
                                  [1, NE]])

                # [PA; v] = max over {Lo, Me} E/O halves (one instruction)
                TT(out=p4_out(0, 3), in0=lmh_pair(0, 1, 0),
                   in1=lmh_pair(0, 1, NE), op=MAX)
                # [u; PC] = min over {Me, Hi} E/O halves (positive
                # sel strides only: negative-stride sel dims fault the HW)
                TT(out=p4_out(1, 1), in0=lmh_pair(1, 1, 0),
                   in1=lmh_pair(1, 1, NE), op=MIN)

                def par2s(s0, step):
                    # P4 sels {s0, s0+step} as [par: col +0/+1][t: 128]
                    return AP(tensor=P4.tensor, offset=P4[:, s0].offset,
                              ap=[p4_pd, [step * R * NE, 2], [NE, R],
                                  [1, 2], [1, 128]])

                def eo3s(s0, step):
                    # LMHT sels as [par=0: E[1:129] | par=1: O[0:128]]
                    return AP(tensor=LMHT.tensor,
                              offset=LMHT[:, s0].offset + 1,
                              ap=[lm_pd, [step * R * WP, 2], [WP, R],
                                  [128, 2], [1, 128]])

                def par2(s0):
                    return AP(tensor=P4.tensor, offset=P4[:, s0].offset,
                              ap=[p4_pd, [NE, R], [1, 2], [1, 128]])

                def eo3(s0):
                    return AP(tensor=LMHT.tensor,
                              offset=LMHT[:, s0].offset + 1,
                              ap=[lm_pd, [WP, R], [128, 2], [1, 128]])

                # Ct and t12 stay separate (3-free-dim limit)
                TT(out=Ct[:], in0=par2(2), in1=eo3(2), op=MIN)
                TT(out=t12[:], in0=par2(3), in1=eo3(1), op=MIN)
                # A = max3(Lo) completion; Bt = B completion
                TT(out=A[:], in0=par2(0), in1=eo3(0), op=MAX)
                TT(out=Bt[:], in0=par2(1), in1=t12[:], op=MAX)

                # deferred stage2 of the previous off2 chunk runs first:
                # its reads of mT2/mU2 must precede this chunk's writers of
                # those tags (inline mT or chain2) in program order
                if pend and pend[0] != k:
                    stage2(pend.pop(0))

                # ---------- final med3(A, B, C) ----------
                if off2(k):
                    # mT/mU go to PE+ACT (chain2, issued below); mV/ot run
                    # as stage2 during the NEXT chunk's stage1 window
                    pend.append(k)
                    cts[k] = Ct_t
                else:
                    mT = wt("mT", R, W)           # dedicated: keeps next
                    mU = wt("mU", R, W, tag="P4")  # chunk's pair writes free
                    mV = wt("mV", R, W, tag="A")   # A dead once mU built
                    o32 = out_pool.tile([128, R * W], f32, name="o32",
                                        tag="o32")
                    o32r = o32.rearrange("p (r w) -> p r w", w=W)
                    ob = o32r[:, :, :]
                    out_ap = AP(tensor=ob.tensor, offset=ob.offset,
                                ap=[list(ob.ap[0]), [W, R], [1, 2], [2, 128]])
                    TT(out=mT[:], in0=A[:], in1=Bt[:], op=MIN)
                    TT(out=mU[:], in0=A[:], in1=Bt[:], op=MAX)
                    TT(out=mV[:], in0=mU[:], in1=Ct[:], op=MIN)
                    if k == K - 1:
                        # tail: the last chunk's final op writes fp32
                        # interleaved directly (1x mode on a tiny chunk),
                        # skipping the ACT hop so the last store starts
                        TT(out=out_ap, in0=mT[:], in1=mV[:], op=MAX)
                    else:
                        ot = ot_pool.tile([128, R * W], f16, name="ot",
                                          tag="ot")
                        otr = ot.rearrange("p (r w) -> p r w", w=W)
                        TT(out=otr[:], in0=mT[:], in1=mV[:], op=MAX)
                        # re-interleave + cast to fp32 on ACT
                        nc.scalar.copy(out=out_ap, in_=otr[:])

                    # ---------- store ----------
                    if k == K - 1:
                        # tail: one DMA for both halves (4D DRAM-side
                        # pattern p = h*64+c -> out[c, h*HH + r0 + r, w])
                        # halves the HWDGE issue cost on the drain path
                        od = AP(tensor=og.tensor, offset=r0 * W,
                                ap=[[HH * W, 2], [H * W, 64], [W, R],
                                    [1, W]])
                        nc.scalar.dma_start(out=od, in_=o32r[:])
                    else:
                        nc.sync.dma_start(out=og[:, r0:r0 + R, :],
                                          in_=o32r[0:64])
                        nc.scalar.dma_start(out=og[:, HH + r0:HH + r0 + R, :],
                                            in_=o32r[64:128])

                # this chunk's chain2 last: its mT2/mU2 writes WAR-order
                # after the stage2 reads issued above (single-buffered)
                if off2(k):
                    chain2(k, A_t, Bt_t)

    nc.compile()
    return nc


def _build_shared(R=16, dtype="float32"):
    """Fallback: fp32 15-op/pixel variant (vertical pair sharing +
    strided horizontal even/odd sharing).  Bit-exact vs the reference."""
    import concourse.bacc as bacc
    import concourse.mybir as mybir
    from concourse.tile import TileContext

    MIN = mybir.AluOpType.min
    MAX = mybir.AluOpType.max
    f32 = mybir.dt.float32

    assert HH % R == 0 and R % 2 == 0
    K = HH // R
    Rh = R // 2

    nc = bacc.Bacc("TRN2", name="median_pool2d_s")
    x = nc.dram_tensor("x", [C, H, W], f32, kind="ExternalInput")
    out = nc.dram_tensor("out", [C, H, W], f32, kind="ExternalOutput")
    xg = x.ap()
    og = out.ap()

    def tt(out_ap, in0, in1, op):
        nc.vector.tensor_tensor(out=out_ap, in0=in0, in1=in1, op=op)

    with TileContext(nc) as tc:
        with (
            tc.tile_pool(name="io_in", bufs=2) as in_pool,
            tc.tile_pool(name="io_out", bufs=1) as out_pool,
            tc.tile_pool(name="work", bufs=1) as w_pool,
        ):
            def wtile(name, rows, width, tag=None):
                t = w_pool.tile([128, rows * width], f32, name=name,
                                tag=tag or name)
                return t.rearrange("p (r w) -> p r w", w=width)

            for k in range(K):
                r0 = k * R
                it = in_pool.tile([128, (R + 2) * WP], f32, name="it",
                                  tag="it")
                it3 = it.rearrange("p (r w) -> p r w", w=WP)
                nc.vector.memset(it3[:, :, 0:WP:WP - 1], 0.0)
                if k == 0:
                    nc.vector.memset(it3[0:64, 0:1, 1:W + 1], 0.0)
                    nc.sync.dma_start(out=it3[0:64, 1:R + 2, 1:W + 1],
                                      in_=xg[:, 0:R + 1, :])
                else:
                    nc.sync.dma_start(out=it3[0:64, :, 1:W + 1],
                                      in_=xg[:, r0 - 1:r0 + R + 1, :])
                if k == K - 1:
                    nc.vector.memset(it3[64:128, R + 1:R + 2, 1:W + 1], 0.0)
                    nc.sync.dma_start(out=it3[64:128, 0:R + 1, 1:W + 1],
                                      in_=xg[:, HH + r0 - 1:H, :])
                else:
                    nc.sync.dma_start(out=it3[64:128, :, 1:W + 1],
                                      in_=xg[:, HH + r0 - 1:HH + r0 + R + 1, :])

                Pm = wtile("Pm", Rh, WP)
                PM = wtile("PM", Rh, WP)
                tt(Pm[:], it3[:, 1:R + 1:2, :], it3[:, 2:R + 2:2, :], MIN)
                tt(PM[:], it3[:, 1:R + 1:2, :], it3[:, 2:R + 2:2, :], MAX)

                Lo3 = wtile("Lo", R, WP)
                Me3 = wtile("Me", R, WP)
                Hi3 = wtile("Hi", R, WP)
                tEv = wtile("tEv", Rh, WP)
                tOv = wtile("tOv", Rh, WP)
                a_e = it3[:, 0:R:2, :]
                a_o = it3[:, 3:R + 2:2, :]
                tt(Lo3[:, 0:R:2], a_e, Pm[:], MIN)
                tt(Hi3[:, 0:R:2], a_e, PM[:], MAX)
                tt(tEv[:], a_e, PM[:], MIN)
                tt(Me3[:, 0:R:2], Pm[:], tEv[:], MAX)
                tt(Lo3[:, 1:R:2], a_o, Pm[:], MIN)
                tt(Hi3[:, 1:R:2], a_o, PM[:], MAX)
                tt(tOv[:], a_o, PM[:], MIN)
                tt(Me3[:, 1:R:2], Pm[:], tOv[:], MAX)

                NP = W // 2 + 1
                PA = wtile("PA", R, NP, tag="Pm")
                PC = wtile("PC", R, NP, tag="PM")
                Um = wtile("Um", R, NP, tag="tEv")
                Vm = wtile("Vm", R, NP, tag="tOv")
                tBe = wtile("tBe", R, W // 2, tag="Pm")
                tBo = wtile("tBo", R, W // 2, tag="PM")
                mA = wtile("mA", R, W)
                mB = wtile("mB", R, W)
                mC = wtile("mC", R, W)

                ev = slice(0, WP, 2)
                od = slice(1, WP, 2)
                tt(PA[:], Lo3[:, :, ev], Lo3[:, :, od], MAX)
                tt(mA[:, :, 0:W:2], PA[:, :, 0:NP - 1], Lo3[:, :, 2:WP:2], MAX)
                tt(mA[:, :, 1:W:2], PA[:, :, 1:NP], Lo3[:, :, 1:WP - 2:2], MAX)

                tt(PC[:], Hi3[:, :, ev], Hi3[:, :, od], MIN)
                tt(mC[:, :, 0:W:2], PC[:, :, 0:NP - 1], Hi3[:, :, 2:WP:2], MIN)
                tt(mC[:, :, 1:W:2], PC[:, :, 1:NP], Hi3[:, :, 1:WP - 2:2], MIN)

                tt(Um[:], Me3[:, :, ev], Me3[:, :, od], MIN)
                tt(Vm[:], Me3[:, :, ev], Me3[:, :, od], MAX)
                tt(tBe[:], Me3[:, :, 2:WP:2], Vm[:, :, 0:NP - 1], MIN)
                tt(mB[:, :, 0:W:2], Um[:, :, 0:NP - 1], tBe[:], MAX)
                tt(tBo[:], Me3[:, :, 1:WP - 2:2], Vm[:, :, 1:NP], MIN)
                tt(mB[:, :, 1:W:2], Um[:, :, 1:NP], tBo[:], MAX)

                mT = wtile("mT", R, W, tag="Lo")
                mU = wtile("mU", R, W, tag="Me")
                mV = wtile("mV", R, W, tag="Hi")
                ot = out_pool.tile([128, R * W], f32, name="ot", tag="ot")
                ot3 = ot.rearrange("p (r w) -> p r w", w=W)
                tt(mT[:], mA[:], mB[:], MIN)
                tt(mU[:], mA[:], mB[:], MAX)
                tt(mV[:], mU[:], mC[:], MIN)
                tt(ot3[:], mT[:], mV[:], MAX)

                nc.sync.dma_start(out=og[:, r0:r0 + R, :], in_=ot3[0:64])
                nc.sync.dma_start(out=og[:, HH + r0:HH + r0 + R, :],
                                  in_=ot3[64:128])

    nc.compile()
    return nc


def _get_nc(kind="f16", **kw):
    key = (kind, tuple(sorted(kw.items())))
    if key not in _CACHE:
        if kind == "f16o":
            kw.setdefault("taper", (2, 6, 8, 12, 12, 12, 12, 12, 12,
                                    12, 12, 12, 2, 2))
            kw.setdefault("off0", 3)
            kw.setdefault("minoff", 12)
            kw.setdefault("keep_tail", 2)
            kw.setdefault("off2_win", (3, 3))
            _CACHE[key] = _build_f16(offload=True, **kw)
        elif kind == "f16":
            _CACHE[key] = _build_f16(**kw)
        else:
            _CACHE[key] = _build_shared(**kw)
    return _CACHE[key]


def _wid():
    eye = np.eye(128, dtype=np.float16)
    return np.concatenate([0.5 * eye, eye, -eye, 2.0 * eye], axis=1)


def _sample_check(x: np.ndarray, out: np.ndarray, k=400_000, seed=0):
    """Exact spot-check of `out` against the true 3x3 zero-padded median
    on k random pixels.  Returns the number of samples whose |err| exceeds
    1e-2 (clean fp16 runs measure <=1e-3; corruption is O(1))."""
    rng = np.random.RandomState(seed)
    b = rng.randint(0, B, k)
    c = rng.randint(0, C, k)
    r = rng.randint(0, H, k)
    w = rng.randint(0, W, k)
    vals = np.empty((k, 9), dtype=np.float32)
    i = 0
    for dr in (-1, 0, 1):
        rr = r + dr
        rok = (rr >= 0) & (rr < H)
        rrc = np.clip(rr, 0, H - 1)
        for dw in (-1, 0, 1):
            ww = w + dw
            ok = rok & (ww >= 0) & (ww < W)
            v = x[b, c, rrc, np.clip(ww, 0, W - 1)]
            vals[:, i] = np.where(ok, v, 0.0)
            i += 1
    med = np.sort(vals, axis=1)[:, 4]
    return int(np.count_nonzero(np.abs(out[b, c, r, w] - med) > 1e-2))


def kernel(x: np.ndarray) -> np.ndarray:
    """MedianPool2d(3x3, s=1, p=1) on 8 NeuronCores, data parallel over
    batch.  fp16 min/max selection network (exact up to fp16 input
    rounding, l2 rel err ~2e-4).  Each run is spot-checked against the
    exact median on 400k random pixels (rare cold-start HW corruption was
    observed once); on failure the kernel is re-run, and as a last resort
    the bit-exact fp32 variant is used."""
    from concourse.bass_utils import run_bass_kernel_spmd

    assert x.shape == (B, C, H, W), x.shape
    x = np.ascontiguousarray(x, dtype=np.float32)
    in_maps = [{"x": x[i]} for i in range(NCORES)]

    def run(nc, extra=None):
        maps = in_maps
        if extra:
            maps = [dict(m, **extra) for m in in_maps]
        res = run_bass_kernel_spmd(nc, maps, core_ids=list(range(NCORES)))
        return np.stack([r["out"] for r in res.results], axis=0)

    try:
        nc = _get_nc("f16o")
        wid = {"wid": _wid()}
        for attempt in range(2):
            out = run(nc, wid)
            if _sample_check(x, out, seed=attempt) <= 3:
                return out
    except Exception:
        pass
    try:
        nc = _get_nc("f16")
        for attempt in range(2):
            out = run(nc)
            if _sample_check(x, out, seed=attempt) <= 3:
                return out
        # persistent corruption: use the bit-exact fp32 variant
        return run(_get_nc("f32", R=16))
    except Exception:
        # fall back to the fp32 builder (bit-exact, ~2x slower)
        return run(_get_nc("f32", R=16))

